# revision 14
# baseline (speedup 1.0000x reference)
"""Trainium2 Bass kernel for nn_Compute_all_u (embedding gather + batched affine dot).

For each voxel v:
    u[v, :] = C[e_v, 0, :] + x_v*C[e_v, 1, :] + y_v*C[e_v, 2, :] + z_v*C[e_v, 3, :]
where e_v = voxels_elements[v], (x,y,z) = all_voxels_centroids[v].

Strategy ("broadcast-R"): shard the ELEMENT TABLE across the 8 cores
(62,500 elements each) and route voxels to the core owning their element.
Each element is then referenced ~16x per core (Poisson(16)), so the device
never needs data-dependent addressing: the host sorts voxels by element and
packs each element's voxels into groups of consecutive slots that share one
(host-repeated) table row; the device streams rows + slot-ordered centroids
and broadcasts each row across its group with stride-0 access patterns.

This removes the SWDGE dma_gather entirely - the v1 kernel was bottlenecked
at ~8.7ns/row of Q7 descriptor generation (1M rows / 4 queues = 2.26ms),
with DMA engines only ~14% busy. Here everything is sequential DMA + DVE.
(Offloading a slice to the Pool engine was tried and REGRESSED: co-running
Pool with DVE halves both engines' SBUF throughput - kept all-DVE.)

MIXED GROUP SIZES cut slot padding: an element with count L gets
floor(L/8) full R=8 groups in region A (plus one more if the remainder
m=L%8 is 5..7), while remainders m=3..4 go to an R=4 region B and m=1..2
to an R=2 region C. Seed-0 slots: 1.08M vs 1.23M for uniform R=8.

Layouts are PLANAR so every DVE operand has innermost stride 1 (the 2x_1P
fp16 perf mode requires step_x=+-1 / 4B alignment on all srcs and dst;
broadcasts live on outer axes where stride 0 is allowed). Each tile's rows
and centroids are packed into ONE dram param (single load per tile):
  tc[t, p, 0:12*cg]        trow planes, dk = d*3+k
  tc[t, p, 12*cg:]         cent planes [j, r, c], j in {x,y,z}
with group g mapped tile-major / partition / column, slots s = g*Rreg + r.

Per tile the 6 fp16 DVE ops (out shape [128, 3, Rreg, cg]) are:
  tmp = X(bcast k) * C1(bcast r);  u  = C0(bcast r) + tmp
  tmp = Y(bcast k) * C2(bcast r);  u += tmp
  tmp = Z(bcast k) * C3(bcast r);  u += tmp

Tiles are SIZE-GRADED (8->24->88 column head ramp, 4x180 mids, 60-column
tail) so the first DVE op waits only on a ~70KB load and the drain is
short; output stores issue from the Activation engine's HWDGE queue so
tile loads (Sync queue) never wait behind them.

Precision: fp16 throughout; measured rel err ~1e-3 vs the f32 reference
(gate 2e-2): values are O(1) normals, u ~ N(0, 4), fp16 eps 9.8e-4.

Host prep per call: one 8M argsort by element, per-core bincount/cumsum to
assign slots, np.repeat to build the group row streams, scatter centroids
into slot-planar order, un-permute outputs. Any voxel whose slot would
exceed a region capacity (seed-0 actual: A 121,418/122,880; B 15,589/16,384;
C 15,601/16,384) falls back to exact host math.
"""

import numpy as np

from concourse import bacc, bass, tile, mybir
from concourse.bass_utils import run_bass_kernel_spmd

N_VOXELS = 8_000_000
N_ELEM = 500_000
N_CORES = 8
EPC = N_ELEM // N_CORES     # 62,500 elements per core
RA, RB, RC = 8, 4, 2

# device tile schedule: (region, n_tiles, group-columns per partition, R, bufs)
# single-tile classes use bufs=1 (no within-class reuse; cross-class loads
# still prefetch from their own pools)
TILES = (
    ("A", 1, 8, RA, 1),     # micro head: compute starts ~0.3us after barrier
    ("A", 1, 24, RA, 1),
    ("A", 1, 88, RA, 1),
    ("A", 2, 360, RA, 2),   # big mids: fewer per-op overheads
    ("B", 1, 128, RB, 1),
    ("C", 1, 128, RC, 1),
    ("A", 2, 60, RA, 2),    # small tail: quick drain
)
CAP = {r: sum(n * 128 * cg for rg, n, cg, _, _ in TILES if rg == r)
       for r in ("A", "B", "C")}          # A: 122,880  B: 16,384  C: 16,384
NSLOT_A = CAP["A"] * RA                   # 983,040
NSLOT_B = CAP["B"] * RB                   # 65,536
NSLOT_C = CAP["C"] * RC                   # 32,768
NSLOT = NSLOT_A + NSLOT_B + NSLOT_C       # 1,081,344 slots per core

f16 = mybir.dt.float16


def build_nc() -> bass.Bass:
    nc = bacc.Bacc("TRN2")
    params = []
    for i, (rg, n, cg, r, _) in enumerate(TILES):
        params.append((
            nc.declare_dram_parameter(
                f"tc{i}", [n, 128, (12 + 3 * r) * cg], f16, isOutput=False
            ),
            nc.declare_dram_parameter(f"out{i}", [n, 128, 3 * r * cg], f16, isOutput=True),
        ))

    mul = mybir.AluOpType.mult
    add = mybir.AluOpType.add

    with tile.TileContext(nc) as tc:
        import contextlib
        with contextlib.ExitStack() as stack:
            pools = [
                stack.enter_context(tc.tile_pool(name=f"io{i}", bufs=b))
                for i, (_, _, _, _, b) in enumerate(TILES)
            ]
            # bufs=1: the DVE is in-order, so tmp WAR across tiles never
            # stalls; double-buffering tmp only mattered for cross-engine use
            tmp_pool = stack.enter_context(tc.tile_pool(name="tmp", bufs=1))

            for i, (rg, n, cg, r, _) in enumerate(TILES):
                tc_in, out = params[i]
                io_pool = pools[i]
                for t in range(n):
                    tc_t = io_pool.tile([128, (12 + 3 * r) * cg], f16, tag=f"tc{i}")
                    nc.sync.dma_start(out=tc_t[:], in_=tc_in[t])

                    u = io_pool.tile([128, 3 * r * cg], f16, tag=f"u{i}")
                    tmp = tmp_pool.tile([128, 3 * r * cg], f16, tag=f"t{i}")

                    tr = tc_t[:, 0:12 * cg].rearrange("p (dk c) -> p dk c", c=cg)
                    cr = tc_t[:, 12 * cg:].rearrange("p (j r c) -> p j r c", r=r, c=cg)
                    ur = u[:].rearrange("p (k r c) -> p k r c", r=r, c=cg)
                    tmr = tmp[:].rearrange("p (k r c) -> p k r c", r=r, c=cg)

                    def rows(d):  # trow planes d*3..d*3+3, bcast over r
                        return tr[:, 3 * d:3 * d + 3, :].unsqueeze(2).to_broadcast(
                            [128, 3, r, cg]
                        )

                    def xyz(j):  # cent plane j, bcast over k
                        return cr[:, j:j + 1, :, :].to_broadcast([128, 3, r, cg])

                    nc.vector.tensor_tensor(out=tmr, in0=xyz(0), in1=rows(1), op=mul)
                    nc.vector.tensor_tensor(out=ur, in0=rows(0), in1=tmr, op=add)
                    nc.vector.tensor_tensor(out=tmr, in0=xyz(1), in1=rows(2), op=mul)
                    nc.vector.tensor_tensor(out=ur, in0=ur, in1=tmr, op=add)
                    nc.vector.tensor_tensor(out=tmr, in0=xyz(2), in1=rows(3), op=mul)
                    nc.vector.tensor_tensor(out=ur, in0=ur, in1=tmr, op=add)

                    # stores ride the Activation engine's HWDGE queue so the
                    # next tiles' loads (Sync queue) never wait behind them
                    nc.scalar.dma_start(out=out[t], in_=u[:])
    nc.finalize()
    return nc


_NC_CACHE: dict = {}


def _get_nc():
    if TILES not in _NC_CACHE:
        _NC_CACHE[TILES] = build_nc()
    return _NC_CACHE[TILES]


def _prep_core(el, vox, coeffs16_c, cent16_full):
    """Build one core's device arrays from its (sorted) local element ids."""
    n = el.shape[0]
    L = np.bincount(el, minlength=EPC)
    q, m = L // RA, L % RA
    a_grp = q + (m >= 5)                             # R=8 groups per element
    b_grp = ((m >= 3) & (m <= 4)).astype(np.int64)   # 0/1 R=4 groups
    c_grp = ((m >= 1) & (m <= 2)).astype(np.int64)   # 0/1 R=2 groups

    a_base = np.zeros(EPC, dtype=np.int64)
    np.cumsum(a_grp[:-1], out=a_base[1:])
    b_base = np.zeros(EPC, dtype=np.int64)
    np.cumsum(b_grp[:-1], out=b_base[1:])
    c_base = np.zeros(EPC, dtype=np.int64)
    np.cumsum(c_grp[:-1], out=c_base[1:])
    run_start = np.zeros(EPC, dtype=np.int64)
    np.cumsum(L[:-1], out=run_start[1:])

    rank = np.arange(n, dtype=np.int64) - run_start[el]
    athr = a_grp[el] * RA                    # slots this element owns in A
    in_a = rank < athr
    in_b = b_grp[el].astype(bool)
    rem = rank - athr
    slot = np.where(
        in_a,
        a_base[el] * RA + rank,
        np.where(
            in_b,
            NSLOT_A + b_base[el] * RB + rem,
            NSLOT_A + NSLOT_B + c_base[el] * RC + rem,
        ),
    )
    ok = np.where(
        in_a,
        slot < NSLOT_A,
        np.where(in_b, slot < NSLOT_A + NSLOT_B, slot < NSLOT),
    )

    def _rows_for(grp, cap, repeat):
        buf = np.zeros((cap, 12), dtype=np.float16)
        if repeat:
            rep = np.repeat(coeffs16_c, grp, axis=0)
        else:
            rep = coeffs16_c[grp.astype(bool)]
        buf[:min(rep.shape[0], cap)] = rep[:cap]
        return buf

    trow = {
        "A": _rows_for(a_grp, CAP["A"], True),
        "B": _rows_for(b_grp, CAP["B"], False),
        "C": _rows_for(c_grp, CAP["C"], False),
    }

    cent_slot = np.zeros((NSLOT, 3), dtype=np.float16)
    cent_slot[slot[ok]] = cent16_full[vox[ok]]

    # slice group-major streams into per-tile-class planar arrays
    reg_R = {"A": RA, "B": RB, "C": RC}
    reg_slot0 = {"A": 0, "B": NSLOT_A, "C": NSLOT_A + NSLOT_B}
    gpos = {"A": 0, "B": 0, "C": 0}
    in_map = {}
    for i, (rg, nt, cg, r, _) in enumerate(TILES):
        ng = nt * 128 * cg
        g0 = gpos[rg]
        rows = trow[rg][g0:g0 + ng]
        s0 = reg_slot0[rg] + g0 * r
        cent = cent_slot[s0:s0 + ng * r]
        gpos[rg] = g0 + ng
        trow_p = rows.reshape(nt, 128, cg, 12).transpose(0, 1, 3, 2).reshape(
            nt, 128, 12 * cg
        )
        cent_p = cent.reshape(nt, 128, cg, r, 3).transpose(0, 1, 4, 3, 2).reshape(
            nt, 128, 3 * r * cg
        )
        in_map[f"tc{i}"] = np.ascontiguousarray(
            np.concatenate([trow_p, cent_p], axis=2)
        )

    return in_map, slot, ok


def _reassemble(results_c):
    """Concatenate per-tile outputs back to [NSLOT, 3] in slot order."""
    parts = {"A": [], "B": [], "C": []}
    for i, (rg, nt, cg, r, _) in enumerate(TILES):
        blk = results_c[f"out{i}"].reshape(nt, 128, 3, r, cg)
        parts[rg].append(
            np.ascontiguousarray(blk.transpose(0, 1, 4, 3, 2)).reshape(-1, 3)
        )
    return np.concatenate(parts["A"] + parts["B"] + parts["C"], axis=0)


def kernel(all_coeffs, all_voxels_centroids, voxels_elements, _trace=False, **run_kwargs):
    nc = _get_nc()
    coeffs12 = np.asarray(all_coeffs, dtype=np.float32).reshape(N_ELEM, 12)
    coeffs16 = coeffs12.astype(np.float16)
    cent_full = np.asarray(all_voxels_centroids, dtype=np.float32)
    cent16 = cent_full.astype(np.float16)
    e_full = np.asarray(voxels_elements).astype(np.int64)

    order = np.argsort(e_full, kind="stable")
    es = e_full[order]
    bounds = np.searchsorted(es, np.arange(N_CORES + 1, dtype=np.int64) * EPC)

    in_maps, metas = [], []
    for c in range(N_CORES):
        lo, hi = int(bounds[c]), int(bounds[c + 1])
        vox = order[lo:hi]
        el = (es[lo:hi] - c * EPC).astype(np.int64)
        m, slot, ok = _prep_core(el, vox, coeffs16[c * EPC:(c + 1) * EPC], cent16)
        in_maps.append(m)
        metas.append((vox, slot, ok))

    res = run_bass_kernel_spmd(
        nc, in_maps, core_ids=list(range(N_CORES)), trace=_trace, **run_kwargs
    )

    full = np.empty((N_VOXELS, 3), dtype=np.float32)
    for c in range(N_CORES):
        vox, slot, ok = metas[c]
        u_slots = _reassemble(res.results[c])
        full[vox[ok]] = u_slots[slot[ok]].astype(np.float32)
        bad = ~ok
        if bad.any():
            vb = vox[bad]
            cf = coeffs12[e_full[vb]].reshape(-1, 4, 3)
            xyz = cent_full[vb]
            full[vb] = cf[:, 0] + np.einsum("nd,ndk->nk", xyz, cf[:, 1:4])
    if _trace:
        return full, res
    return full


# revision 16
# speedup vs baseline: 1.0312x; 1.0312x over previous
"""Trainium2 Bass kernel for nn_Compute_all_u (embedding gather + batched affine dot).

For each voxel v:
    u[v, :] = C[e_v, 0, :] + x_v*C[e_v, 1, :] + y_v*C[e_v, 2, :] + z_v*C[e_v, 3, :]
where e_v = voxels_elements[v], (x,y,z) = all_voxels_centroids[v].

Strategy ("broadcast-R"): shard the ELEMENT TABLE across the 8 cores
(62,500 elements each) and route voxels to the core owning their element.
Each element is then referenced ~16x per core (Poisson(16)), so the device
never needs data-dependent addressing: the host sorts voxels by element and
packs each element's voxels into groups of consecutive slots that share one
(host-repeated) table row; the device streams rows + slot-ordered centroids
and broadcasts each row across its group with stride-0 access patterns.

This removes the SWDGE dma_gather entirely - the v1 kernel was bottlenecked
at ~8.7ns/row of Q7 descriptor generation (1M rows / 4 queues = 2.26ms),
with DMA engines only ~14% busy. Here everything is sequential DMA + DVE.
(Offloading a slice to the Pool engine was tried and REGRESSED: co-running
Pool with DVE halves both engines' SBUF throughput - kept all-DVE.)

MIXED GROUP SIZES cut slot padding: an element with count L gets
floor(L/8) full R=8 groups in region A (plus one more if the remainder
m=L%8 is 5..7), while remainders m=3..4 go to an R=4 region B and m=1..2
to an R=2 region C. Seed-0 slots: 1.08M vs 1.23M for uniform R=8.

Layouts are PLANAR so every DVE operand has innermost stride 1 (the 2x_1P
fp16 perf mode requires step_x=+-1 / 4B alignment on all srcs and dst;
broadcasts live on outer axes where stride 0 is allowed). Each tile's rows
and centroids are packed into ONE dram param (single load per tile):
  tc[t, p, 0:12*cg]        trow planes, dk = d*3+k
  tc[t, p, 12*cg:]         cent planes [j, r, c], j in {x,y,z}
with group g mapped tile-major / partition / column, slots s = g*Rreg + r.

Per tile the 6 fp16 DVE ops (out shape [128, 3, Rreg, cg]) are:
  tmp = X(bcast k) * C1(bcast r);  u  = C0(bcast r) + tmp
  tmp = Y(bcast k) * C2(bcast r);  u += tmp
  tmp = Z(bcast k) * C3(bcast r);  u += tmp

Tiles are SIZE-GRADED (8->24->88 column head ramp, 4x180 mids, 60-column
tail) so the first DVE op waits only on a ~70KB load and the drain is
short; output stores issue from the Activation engine's HWDGE queue so
tile loads (Sync queue) never wait behind them.

Precision: fp16 throughout; measured rel err ~1e-3 vs the f32 reference
(gate 2e-2): values are O(1) normals, u ~ N(0, 4), fp16 eps 9.8e-4.

Host prep per call: one 8M argsort by element, per-core bincount/cumsum to
assign slots, np.repeat to build the group row streams, scatter centroids
into slot-planar order, un-permute outputs. Any voxel whose slot would
exceed a region capacity (seed-0 actual: A 121,418/122,880; B 15,589/16,384;
C 15,601/16,384) falls back to exact host math.
"""

import numpy as np

from concourse import bacc, bass, tile, mybir
from concourse.bass_utils import run_bass_kernel_spmd

N_VOXELS = 8_000_000
N_ELEM = 500_000
N_CORES = 8
EPC = N_ELEM // N_CORES     # 62,500 elements per core
RA, RB, RC = 8, 4, 2

# device tile schedule: (region, n_tiles, group-columns per partition, R, bufs)
TILES = (
    ("A", 1, 8, RA, 2),     # micro head: compute starts ~0.3us after barrier
    ("A", 1, 24, RA, 2),
    ("A", 1, 88, RA, 2),
    ("A", 4, 180, RA, 3),   # big mids: fewer per-op overheads
    ("B", 1, 128, RB, 2),
    ("C", 1, 128, RC, 2),
    ("A", 2, 60, RA, 2),    # small tail: quick drain
)
CAP = {r: sum(n * 128 * cg for rg, n, cg, _, _ in TILES if rg == r)
       for r in ("A", "B", "C")}          # A: 122,880  B: 16,384  C: 16,384
NSLOT_A = CAP["A"] * RA                   # 983,040
NSLOT_B = CAP["B"] * RB                   # 65,536
NSLOT_C = CAP["C"] * RC                   # 32,768
NSLOT = NSLOT_A + NSLOT_B + NSLOT_C       # 1,081,344 slots per core

f16 = mybir.dt.float16


def build_nc() -> bass.Bass:
    nc = bacc.Bacc("TRN2")
    params = []
    for i, (rg, n, cg, r, _) in enumerate(TILES):
        params.append((
            nc.declare_dram_parameter(
                f"tc{i}", [n, 128, (12 + 3 * r) * cg], f16, isOutput=False
            ),
            nc.declare_dram_parameter(f"out{i}", [n, 128, 3 * r * cg], f16, isOutput=True),
        ))

    mul = mybir.AluOpType.mult
    add = mybir.AluOpType.add

    with tile.TileContext(nc) as tc:
        import contextlib
        with contextlib.ExitStack() as stack:
            pools = [
                stack.enter_context(tc.tile_pool(name=f"io{i}", bufs=b))
                for i, (_, _, _, _, b) in enumerate(TILES)
            ]
            tmp_pool = stack.enter_context(tc.tile_pool(name="tmp", bufs=2))

            for i, (rg, n, cg, r, _) in enumerate(TILES):
                tc_in, out = params[i]
                io_pool = pools[i]
                for t in range(n):
                    tc_t = io_pool.tile([128, (12 + 3 * r) * cg], f16, tag=f"tc{i}")
                    nc.sync.dma_start(out=tc_t[:], in_=tc_in[t])

                    u = io_pool.tile([128, 3 * r * cg], f16, tag=f"u{i}")
                    tmp = tmp_pool.tile([128, 3 * r * cg], f16, tag=f"t{i}")

                    tr = tc_t[:, 0:12 * cg].rearrange("p (dk c) -> p dk c", c=cg)
                    cr = tc_t[:, 12 * cg:].rearrange("p (j r c) -> p j r c", r=r, c=cg)
                    ur = u[:].rearrange("p (k r c) -> p k r c", r=r, c=cg)
                    tmr = tmp[:].rearrange("p (k r c) -> p k r c", r=r, c=cg)

                    def rows(d):  # trow planes d*3..d*3+3, bcast over r
                        return tr[:, 3 * d:3 * d + 3, :].unsqueeze(2).to_broadcast(
                            [128, 3, r, cg]
                        )

                    def xyz(j):  # cent plane j, bcast over k
                        return cr[:, j:j + 1, :, :].to_broadcast([128, 3, r, cg])

                    nc.vector.tensor_tensor(out=tmr, in0=xyz(0), in1=rows(1), op=mul)
                    nc.vector.tensor_tensor(out=ur, in0=rows(0), in1=tmr, op=add)
                    nc.vector.tensor_tensor(out=tmr, in0=xyz(1), in1=rows(2), op=mul)
                    nc.vector.tensor_tensor(out=ur, in0=ur, in1=tmr, op=add)
                    nc.vector.tensor_tensor(out=tmr, in0=xyz(2), in1=rows(3), op=mul)
                    nc.vector.tensor_tensor(out=ur, in0=ur, in1=tmr, op=add)

                    # stores ride the Activation engine's HWDGE queue so the
                    # next tiles' loads (Sync queue) never wait behind them
                    nc.scalar.dma_start(out=out[t], in_=u[:])
    nc.finalize()
    return nc


_NC_CACHE: dict = {}


def _get_nc():
    if TILES not in _NC_CACHE:
        _NC_CACHE[TILES] = build_nc()
    return _NC_CACHE[TILES]


def _prep_core(el, vox, coeffs16_c, cent16_full):
    """Build one core's device arrays from its (sorted) local element ids."""
    n = el.shape[0]
    L = np.bincount(el, minlength=EPC)
    q, m = L // RA, L % RA
    a_grp = q + (m >= 5)                             # R=8 groups per element
    b_grp = ((m >= 3) & (m <= 4)).astype(np.int64)   # 0/1 R=4 groups
    c_grp = ((m >= 1) & (m <= 2)).astype(np.int64)   # 0/1 R=2 groups

    a_base = np.zeros(EPC, dtype=np.int64)
    np.cumsum(a_grp[:-1], out=a_base[1:])
    b_base = np.zeros(EPC, dtype=np.int64)
    np.cumsum(b_grp[:-1], out=b_base[1:])
    c_base = np.zeros(EPC, dtype=np.int64)
    np.cumsum(c_grp[:-1], out=c_base[1:])
    run_start = np.zeros(EPC, dtype=np.int64)
    np.cumsum(L[:-1], out=run_start[1:])

    rank = np.arange(n, dtype=np.int64) - run_start[el]
    athr = a_grp[el] * RA                    # slots this element owns in A
    in_a = rank < athr
    in_b = b_grp[el].astype(bool)
    rem = rank - athr
    slot = np.where(
        in_a,
        a_base[el] * RA + rank,
        np.where(
            in_b,
            NSLOT_A + b_base[el] * RB + rem,
            NSLOT_A + NSLOT_B + c_base[el] * RC + rem,
        ),
    )
    ok = np.where(
        in_a,
        slot < NSLOT_A,
        np.where(in_b, slot < NSLOT_A + NSLOT_B, slot < NSLOT),
    )

    def _rows_for(grp, cap, repeat):
        buf = np.zeros((cap, 12), dtype=np.float16)
        if repeat:
            rep = np.repeat(coeffs16_c, grp, axis=0)
        else:
            rep = coeffs16_c[grp.astype(bool)]
        buf[:min(rep.shape[0], cap)] = rep[:cap]
        return buf

    trow = {
        "A": _rows_for(a_grp, CAP["A"], True),
        "B": _rows_for(b_grp, CAP["B"], False),
        "C": _rows_for(c_grp, CAP["C"], False),
    }

    cent_slot = np.zeros((NSLOT, 3), dtype=np.float16)
    cent_slot[slot[ok]] = cent16_full[vox[ok]]

    # slice group-major streams into per-tile-class planar arrays
    reg_R = {"A": RA, "B": RB, "C": RC}
    reg_slot0 = {"A": 0, "B": NSLOT_A, "C": NSLOT_A + NSLOT_B}
    gpos = {"A": 0, "B": 0, "C": 0}
    in_map = {}
    for i, (rg, nt, cg, r, _) in enumerate(TILES):
        ng = nt * 128 * cg
        g0 = gpos[rg]
        rows = trow[rg][g0:g0 + ng]
        s0 = reg_slot0[rg] + g0 * r
        cent = cent_slot[s0:s0 + ng * r]
        gpos[rg] = g0 + ng
        trow_p = rows.reshape(nt, 128, cg, 12).transpose(0, 1, 3, 2).reshape(
            nt, 128, 12 * cg
        )
        cent_p = cent.reshape(nt, 128, cg, r, 3).transpose(0, 1, 4, 3, 2).reshape(
            nt, 128, 3 * r * cg
        )
        in_map[f"tc{i}"] = np.ascontiguousarray(
            np.concatenate([trow_p, cent_p], axis=2)
        )

    return in_map, slot, ok


def _reassemble(results_c):
    """Concatenate per-tile outputs back to [NSLOT, 3] in slot order."""
    parts = {"A": [], "B": [], "C": []}
    for i, (rg, nt, cg, r, _) in enumerate(TILES):
        blk = results_c[f"out{i}"].reshape(nt, 128, 3, r, cg)
        parts[rg].append(
            np.ascontiguousarray(blk.transpose(0, 1, 4, 3, 2)).reshape(-1, 3)
        )
    return np.concatenate(parts["A"] + parts["B"] + parts["C"], axis=0)


def kernel(all_coeffs, all_voxels_centroids, voxels_elements, _trace=False, **run_kwargs):
    nc = _get_nc()
    coeffs12 = np.asarray(all_coeffs, dtype=np.float32).reshape(N_ELEM, 12)
    coeffs16 = coeffs12.astype(np.float16)
    cent_full = np.asarray(all_voxels_centroids, dtype=np.float32)
    cent16 = cent_full.astype(np.float16)
    e_full = np.asarray(voxels_elements).astype(np.int64)

    order = np.argsort(e_full, kind="stable")
    es = e_full[order]
    bounds = np.searchsorted(es, np.arange(N_CORES + 1, dtype=np.int64) * EPC)

    in_maps, metas = [], []
    for c in range(N_CORES):
        lo, hi = int(bounds[c]), int(bounds[c + 1])
        vox = order[lo:hi]
        el = (es[lo:hi] - c * EPC).astype(np.int64)
        m, slot, ok = _prep_core(el, vox, coeffs16[c * EPC:(c + 1) * EPC], cent16)
        in_maps.append(m)
        metas.append((vox, slot, ok))

    res = run_bass_kernel_spmd(
        nc, in_maps, core_ids=list(range(N_CORES)), trace=_trace, **run_kwargs
    )

    full = np.empty((N_VOXELS, 3), dtype=np.float32)
    for c in range(N_CORES):
        vox, slot, ok = metas[c]
        u_slots = _reassemble(res.results[c])
        full[vox[ok]] = u_slots[slot[ok]].astype(np.float32)
        bad = ~ok
        if bad.any():
            vb = vox[bad]
            cf = coeffs12[e_full[vb]].reshape(-1, 4, 3)
            xyz = cent_full[vb]
            full[vb] = cf[:, 0] + np.einsum("nd,ndk->nk", xyz, cf[:, 1:4])
    if _trace:
        return full, res
    return full


# revision 17
# speedup vs baseline: 1.0599x; 1.0278x over previous
"""Trainium2 Bass kernel for nn_Compute_all_u (embedding gather + batched affine dot).

For each voxel v:
    u[v, :] = C[e_v, 0, :] + x_v*C[e_v, 1, :] + y_v*C[e_v, 2, :] + z_v*C[e_v, 3, :]
where e_v = voxels_elements[v], (x,y,z) = all_voxels_centroids[v].

Strategy ("broadcast-R"): shard the ELEMENT TABLE across the 8 cores
(62,500 elements each) and route voxels to the core owning their element.
Each element is then referenced ~16x per core (Poisson(16)), so the device
never needs data-dependent addressing: the host sorts voxels by element and
packs each element's voxels into groups of consecutive slots that share one
(host-repeated) table row; the device streams rows + slot-ordered centroids
and broadcasts each row across its group with stride-0 access patterns.

This removes the SWDGE dma_gather entirely - the v1 kernel was bottlenecked
at ~8.7ns/row of Q7 descriptor generation (1M rows / 4 queues = 2.26ms),
with DMA engines only ~14% busy. Here everything is sequential DMA + DVE.
(Offloading a slice to the Pool engine was tried and REGRESSED: co-running
Pool with DVE halves both engines' SBUF throughput - kept all-DVE.)

MIXED GROUP SIZES cut slot padding to ~3%: an element with count L gets
floor(L/8) full R=8 groups (plus one more if the remainder m=L%8 is 7),
while remainders m=5..6 go to an R=6 region, m=3..4 to R=4, m=2 to R=2 and
m=1 to R=1. Region capacities sit just above the actual seed-0 per-core
maxima; any overflow voxel falls back to exact host math (correctness never
depends on the caps). Seed-0 slots: 1.03M vs 1.23M for uniform R=8.

Layouts are PLANAR so every DVE operand has innermost stride 1 (the 2x_1P
fp16 perf mode requires step_x=+-1 / 4B alignment on all srcs and dst -
hence even column counts; broadcasts live on outer axes where stride 0 is
allowed). Each tile's rows and centroids are packed into ONE dram param
(single load per tile):
  tc[t, p, 0:12*cg]        trow planes, dk = d*3+k
  tc[t, p, 12*cg:]         cent planes [j, r, c], j in {x,y,z}
with group g mapped tile-major / partition / column, slots s = g*Rreg + r.

Per tile the 6 fp16 DVE ops (out shape [128, 3, Rreg, cg]) are:
  tmp = X(bcast k) * C1(bcast r);  u  = C0(bcast r) + tmp
  tmp = Y(bcast k) * C2(bcast r);  u += tmp
  tmp = Z(bcast k) * C3(bcast r);  u += tmp

Tiles are SIZE-GRADED: an 8->24->88 column head ramp lets the first DVE op
wait only on a ~74KB load, and the schedule ends on an 8-column micro tile
so the final store (~49KB) drains fast before the exit barrier. Output
stores issue from the Activation engine's HWDGE queue so tile loads (Sync
queue) never wait behind them. tmp uses a single buffer - the DVE is
in-order so tmp write-after-read never stalls.

Precision: fp16 throughout; measured rel err ~1e-3 vs the f32 reference
(gate 2e-2): values are O(1) normals, u ~ N(0, 4), fp16 eps 9.8e-4.

Host prep per call: one 8M argsort by element, per-core bincount/cumsum to
assign slots, np.repeat to build the group row streams, scatter centroids
into slot-planar order, un-permute outputs.
"""

import numpy as np

from concourse import bacc, bass, tile, mybir
from concourse.bass_utils import run_bass_kernel_spmd

N_VOXELS = 8_000_000
N_ELEM = 500_000
N_CORES = 8
EPC = N_ELEM // N_CORES     # 62,500 elements per core

# regions in slot order; R per region (A holds full groups + m==7)
REGION_R = {"A": 8, "S": 6, "B": 4, "C": 2, "D": 1}

# device tile schedule: (region, n_tiles, group-columns per partition, bufs)
# column counts are EVEN (4B alignment for the DVE 2x fp16 mode)
TILES = (
    ("A", 1, 8, 2),      # micro head: compute starts ~0.3us after barrier
    ("A", 1, 24, 2),
    ("A", 1, 88, 2),
    ("A", 4, 160, 3),    # big mids
    ("S", 1, 126, 2),    # m=5,6 (seed-0 max 15,998 / cap 16,128)
    ("B", 1, 124, 2),    # m=3,4 (15,589 / 15,872)
    ("C", 1, 62, 2),     # m=2   (7,846 / 7,936)
    ("D", 1, 62, 2),     # m=1   (7,894 / 7,936)
    ("A", 1, 58, 2),
    ("A", 1, 8, 2),      # micro tail: final store is tiny
)
# A capacity 826 cols = 105,728 groups (seed-0 max 105,561)
CAP = {r: sum(n * 128 * cg for rg, n, cg, _ in TILES if rg == r) for r in REGION_R}
NS = {r: CAP[r] * REGION_R[r] for r in REGION_R}
_order = list(REGION_R)
SLOT0 = {}
_acc = 0
for _r in _order:
    SLOT0[_r] = _acc
    _acc += NS[_r]
NSLOT = _acc               # 1,029,888 slots per core

f16 = mybir.dt.float16


def build_nc() -> bass.Bass:
    nc = bacc.Bacc("TRN2")
    params = []
    for i, (rg, n, cg, _) in enumerate(TILES):
        r = REGION_R[rg]
        params.append((
            nc.declare_dram_parameter(
                f"tc{i}", [n, 128, (12 + 3 * r) * cg], f16, isOutput=False
            ),
            nc.declare_dram_parameter(f"out{i}", [n, 128, 3 * r * cg], f16, isOutput=True),
        ))

    mul = mybir.AluOpType.mult
    add = mybir.AluOpType.add

    with tile.TileContext(nc) as tc:
        import contextlib
        with contextlib.ExitStack() as stack:
            pools = [
                stack.enter_context(tc.tile_pool(name=f"io{i}", bufs=b))
                for i, (_, _, _, b) in enumerate(TILES)
            ]
            # bufs=1: tmp is written/read only by the in-order DVE, so
            # write-after-read across tiles never stalls
            tmp_pool = stack.enter_context(tc.tile_pool(name="tmp", bufs=1))

            for i, (rg, n, cg, _) in enumerate(TILES):
                r = REGION_R[rg]
                tc_in, out = params[i]
                io_pool = pools[i]
                for t in range(n):
                    tc_t = io_pool.tile([128, (12 + 3 * r) * cg], f16, tag=f"tc{i}")
                    nc.sync.dma_start(out=tc_t[:], in_=tc_in[t])

                    u = io_pool.tile([128, 3 * r * cg], f16, tag=f"u{i}")
                    tmp = tmp_pool.tile([128, 3 * r * cg], f16, tag=f"t{i}")

                    tr = tc_t[:, 0:12 * cg].rearrange("p (dk c) -> p dk c", c=cg)
                    cr = tc_t[:, 12 * cg:].rearrange("p (j r c) -> p j r c", r=r, c=cg)
                    ur = u[:].rearrange("p (k r c) -> p k r c", r=r, c=cg)
                    tmr = tmp[:].rearrange("p (k r c) -> p k r c", r=r, c=cg)

                    def rows(d):  # trow planes d*3..d*3+3, bcast over r
                        return tr[:, 3 * d:3 * d + 3, :].unsqueeze(2).to_broadcast(
                            [128, 3, r, cg]
                        )

                    def xyz(j):  # cent plane j, bcast over k
                        return cr[:, j:j + 1, :, :].to_broadcast([128, 3, r, cg])

                    nc.vector.tensor_tensor(out=tmr, in0=xyz(0), in1=rows(1), op=mul)
                    nc.vector.tensor_tensor(out=ur, in0=rows(0), in1=tmr, op=add)
                    nc.vector.tensor_tensor(out=tmr, in0=xyz(1), in1=rows(2), op=mul)
                    nc.vector.tensor_tensor(out=ur, in0=ur, in1=tmr, op=add)
                    nc.vector.tensor_tensor(out=tmr, in0=xyz(2), in1=rows(3), op=mul)
                    nc.vector.tensor_tensor(out=ur, in0=ur, in1=tmr, op=add)

                    # stores ride the Activation engine's HWDGE queue so the
                    # next tiles' loads (Sync queue) never wait behind them
                    nc.scalar.dma_start(out=out[t], in_=u[:])
    nc.finalize()
    return nc


_NC_CACHE: dict = {}


def _get_nc():
    if TILES not in _NC_CACHE:
        _NC_CACHE[TILES] = build_nc()
    return _NC_CACHE[TILES]


def _prep_core(el, vox, coeffs16_c, cent16_full):
    """Build one core's device arrays from its (sorted) local element ids."""
    n = el.shape[0]
    L = np.bincount(el, minlength=EPC)
    q, m = L // 8, L % 8
    grp = {
        "A": q + (m == 7),
        "S": ((m == 5) | (m == 6)).astype(np.int64),
        "B": ((m == 3) | (m == 4)).astype(np.int64),
        "C": (m == 2).astype(np.int64),
        "D": (m == 1).astype(np.int64),
    }
    base = {}
    for r in _order:
        b = np.zeros(EPC, dtype=np.int64)
        np.cumsum(grp[r][:-1], out=b[1:])
        base[r] = b
    run_start = np.zeros(EPC, dtype=np.int64)
    np.cumsum(L[:-1], out=run_start[1:])

    rank = np.arange(n, dtype=np.int64) - run_start[el]
    athr = grp["A"][el] * 8              # slots this element owns in A
    in_a = rank < athr
    rem = rank - athr
    m_el = m[el]
    # remainder region index into _order (unused entries default to A=0)
    ridx = np.select(
        [(m_el == 5) | (m_el == 6), (m_el == 3) | (m_el == 4), m_el == 2, m_el == 1],
        [1, 2, 3, 4],
        default=0,
    )
    r_arr = np.array([REGION_R[r] for r in _order])
    slot0_arr = np.array([SLOT0[r] for r in _order])
    end_arr = np.array([SLOT0[r] + NS[r] for r in _order])
    base_stack = np.stack([base[r] for r in _order])     # [5, EPC]
    rem_slot = slot0_arr[ridx] + base_stack[ridx, el] * r_arr[ridx] + rem
    slot = np.where(in_a, base["A"][el] * 8 + rank, rem_slot)
    ok = np.where(in_a, slot < NS["A"], slot < end_arr[ridx])

    trow = {}
    for r in _order:
        buf = np.zeros((CAP[r], 12), dtype=np.float16)
        if r == "A":
            rep = np.repeat(coeffs16_c, grp[r], axis=0)
        else:
            rep = coeffs16_c[grp[r].astype(bool)]
        buf[:min(rep.shape[0], CAP[r])] = rep[:CAP[r]]
        trow[r] = buf

    cent_slot = np.zeros((NSLOT, 3), dtype=np.float16)
    cent_slot[slot[ok]] = cent16_full[vox[ok]]

    # slice group-major streams into per-tile-class planar arrays
    gpos = {r: 0 for r in _order}
    in_map = {}
    for i, (rg, nt, cg, _) in enumerate(TILES):
        r = REGION_R[rg]
        ng = nt * 128 * cg
        g0 = gpos[rg]
        rows = trow[rg][g0:g0 + ng]
        s0 = SLOT0[rg] + g0 * r
        cent = cent_slot[s0:s0 + ng * r]
        gpos[rg] = g0 + ng
        trow_p = rows.reshape(nt, 128, cg, 12).transpose(0, 1, 3, 2).reshape(
            nt, 128, 12 * cg
        )
        cent_p = cent.reshape(nt, 128, cg, r, 3).transpose(0, 1, 4, 3, 2).reshape(
            nt, 128, 3 * r * cg
        )
        in_map[f"tc{i}"] = np.ascontiguousarray(
            np.concatenate([trow_p, cent_p], axis=2)
        )

    return in_map, slot, ok


def _reassemble(results_c):
    """Concatenate per-tile outputs back to [NSLOT, 3] in slot order."""
    parts = {r: [] for r in _order}
    for i, (rg, nt, cg, _) in enumerate(TILES):
        r = REGION_R[rg]
        blk = results_c[f"out{i}"].reshape(nt, 128, 3, r, cg)
        parts[rg].append(
            np.ascontiguousarray(blk.transpose(0, 1, 4, 3, 2)).reshape(-1, 3)
        )
    return np.concatenate([p for r in _order for p in parts[r]], axis=0)


def kernel(all_coeffs, all_voxels_centroids, voxels_elements, _trace=False, **run_kwargs):
    nc = _get_nc()
    coeffs12 = np.asarray(all_coeffs, dtype=np.float32).reshape(N_ELEM, 12)
    coeffs16 = coeffs12.astype(np.float16)
    cent_full = np.asarray(all_voxels_centroids, dtype=np.float32)
    cent16 = cent_full.astype(np.float16)
    e_full = np.asarray(voxels_elements).astype(np.int64)

    order = np.argsort(e_full, kind="stable")
    es = e_full[order]
    bounds = np.searchsorted(es, np.arange(N_CORES + 1, dtype=np.int64) * EPC)

    in_maps, metas = [], []
    for c in range(N_CORES):
        lo, hi = int(bounds[c]), int(bounds[c + 1])
        vox = order[lo:hi]
        el = (es[lo:hi] - c * EPC).astype(np.int64)
        m, slot, ok = _prep_core(el, vox, coeffs16[c * EPC:(c + 1) * EPC], cent16)
        in_maps.append(m)
        metas.append((vox, slot, ok))

    res = run_bass_kernel_spmd(
        nc, in_maps, core_ids=list(range(N_CORES)), trace=_trace, **run_kwargs
    )

    full = np.empty((N_VOXELS, 3), dtype=np.float32)
    for c in range(N_CORES):
        vox, slot, ok = metas[c]
        u_slots = _reassemble(res.results[c])
        full[vox[ok]] = u_slots[slot[ok]].astype(np.float32)
        bad = ~ok
        if bad.any():
            vb = vox[bad]
            cf = coeffs12[e_full[vb]].reshape(-1, 4, 3)
            xyz = cent_full[vb]
            full[vb] = cf[:, 0] + np.einsum("nd,ndk->nk", xyz, cf[:, 1:4])
    if _trace:
        return full, res
    return full


# revision 20
# speedup vs baseline: 1.0643x; 1.0041x over previous
"""Trainium2 Bass kernel for nn_Compute_all_u (embedding gather + batched affine dot).

For each voxel v:
    u[v, :] = C[e_v, 0, :] + x_v*C[e_v, 1, :] + y_v*C[e_v, 2, :] + z_v*C[e_v, 3, :]
where e_v = voxels_elements[v], (x,y,z) = all_voxels_centroids[v].

Strategy ("broadcast-R"): shard the ELEMENT TABLE across the 8 cores
(62,500 elements each) and route voxels to the core owning their element.
Each element is then referenced ~16x per core (Poisson(16)), so the device
never needs data-dependent addressing: the host sorts voxels by element and
packs each element's voxels into groups of consecutive slots that share one
(host-repeated) table row; the device streams rows + slot-ordered centroids
and broadcasts each row across its group with stride-0 access patterns.

This removes the SWDGE dma_gather entirely - the v1 kernel was bottlenecked
at ~8.7ns/row of Q7 descriptor generation (1M rows / 4 queues = 2.26ms),
with DMA engines only ~14% busy. Here everything is sequential DMA + DVE.
(Offloading a slice to the Pool engine was tried and REGRESSED: co-running
Pool with DVE halves both engines' SBUF throughput - kept all-DVE.)

MIXED GROUP SIZES cut slot padding to ~3%: an element with count L gets
floor(L/8) full R=8 groups (plus one more if the remainder m=L%8 is 7),
while remainders m=5..6 go to an R=6 region, m=3..4 to R=4, m=2 to R=2 and
m=1 to R=1. Region capacities sit just above the actual seed-0 per-core
maxima; any overflow voxel falls back to exact host math (correctness never
depends on the caps). Seed-0 slots: 1.03M vs 1.23M for uniform R=8.

Layouts are PLANAR so every DVE operand has innermost stride 1 (the 2x_1P
fp16 perf mode requires step_x=+-1 / 4B alignment on all srcs and dst -
hence even column counts; broadcasts live on outer axes where stride 0 is
allowed). Each tile's rows and centroids are packed into ONE dram param
(single load per tile):
  tc[t, p, 0:12*cg]        trow planes, dk = d*3+k
  tc[t, p, 12*cg:]         cent planes [j, r, c], j in {x,y,z}
with group g mapped tile-major / partition / column, slots s = g*Rreg + r.

Per tile the 6 fp16 DVE ops (out shape [128, 3, Rreg, cg]) are:
  tmp = X(bcast k) * C1(bcast r);  u  = C0(bcast r) + tmp
  tmp = Y(bcast k) * C2(bcast r);  u += tmp
  tmp = Z(bcast k) * C3(bcast r);  u += tmp

Tiles are SIZE-GRADED: an 8->24->88 column head ramp lets the first DVE op
wait only on a ~74KB load, and the schedule ends on an 8-column micro tile
so the final store (~49KB) drains fast before the exit barrier. Output
stores issue from the Activation engine's HWDGE queue so tile loads (Sync
queue) never wait behind them. tmp uses a single buffer - the DVE is
in-order so tmp write-after-read never stalls.

Precision: fp16 throughout; measured rel err ~1e-3 vs the f32 reference
(gate 2e-2): values are O(1) normals, u ~ N(0, 4), fp16 eps 9.8e-4.

Host prep per call: one 8M argsort by element, per-core bincount/cumsum to
assign slots, np.repeat to build the group row streams, scatter centroids
into slot-planar order, un-permute outputs.
"""

import numpy as np

from concourse import bacc, bass, tile, mybir
from concourse.bass_utils import run_bass_kernel_spmd

N_VOXELS = 8_000_000
N_ELEM = 500_000
N_CORES = 8
EPC = N_ELEM // N_CORES     # 62,500 elements per core

# regions in slot order; A holds the floor(L/8) full groups, region Mk the
# size-k remainder of every element with L%8 == k (zero interior padding)
REGION_R = {"A": 8, "M7": 7, "M6": 6, "M5": 5, "M4": 4, "M3": 3, "M2": 2, "M1": 1}

# device tile schedule: (region, n_tiles, group-columns per partition, bufs)
# column counts are EVEN (4B alignment for the DVE 2x fp16 mode); bufs=1 on
# single-tile classes (a once-used tag never reuses its buffer)
TILES = (
    ("A", 1, 8, 1),      # micro head: compute starts ~0.3us after barrier
    ("A", 1, 24, 1),
    ("A", 1, 88, 1),
    ("A", 4, 150, 4),    # big mids
    ("M7", 1, 64, 1),    # seed-0 max 8,018 / cap 8,192
    ("M6", 1, 64, 1),    # 8,030 / 8,192
    ("M5", 1, 64, 1),    # 7,986 / 8,192
    ("M4", 1, 62, 1),    # 7,879 / 7,936
    ("M3", 1, 62, 1),    # 7,780 / 7,936
    ("M2", 1, 62, 1),    # 7,846 / 7,936
    ("M1", 1, 62, 1),    # 7,894 / 7,936
    ("A", 1, 38, 1),
    ("A", 1, 8, 1),      # micro tail: final store is tiny
)
# A capacity 766 cols = 98,048 groups (seed-0 max 97,676)
CAP = {r: sum(n * 128 * cg for rg, n, cg, _ in TILES if rg == r) for r in REGION_R}
NS = {r: CAP[r] * REGION_R[r] for r in REGION_R}
_order = list(REGION_R)
SLOT0 = {}
_acc = 0
for _r in _order:
    SLOT0[_r] = _acc
    _acc += NS[_r]
NSLOT = _acc               # 1,029,888 slots per core

f16 = mybir.dt.float16


def build_nc() -> bass.Bass:
    nc = bacc.Bacc("TRN2")
    params = []
    for i, (rg, n, cg, _) in enumerate(TILES):
        r = REGION_R[rg]
        params.append((
            nc.declare_dram_parameter(
                f"tc{i}", [n, 128, (12 + 3 * r) * cg], f16, isOutput=False
            ),
            nc.declare_dram_parameter(f"out{i}", [n, 128, 3 * r * cg], f16, isOutput=True),
        ))

    mul = mybir.AluOpType.mult
    add = mybir.AluOpType.add

    with tile.TileContext(nc) as tc:
        import contextlib
        with contextlib.ExitStack() as stack:
            pools = [
                stack.enter_context(tc.tile_pool(name=f"io{i}", bufs=b))
                for i, (_, _, _, b) in enumerate(TILES)
            ]
            # bufs=1: tmp is written/read only by the in-order DVE, so
            # write-after-read across tiles never stalls
            tmp_pool = stack.enter_context(tc.tile_pool(name="tmp", bufs=1))

            for i, (rg, n, cg, _) in enumerate(TILES):
                r = REGION_R[rg]
                tc_in, out = params[i]
                io_pool = pools[i]
                for t in range(n):
                    tc_t = io_pool.tile([128, (12 + 3 * r) * cg], f16, tag=f"tc{i}")
                    nc.sync.dma_start(out=tc_t[:], in_=tc_in[t])

                    u = io_pool.tile([128, 3 * r * cg], f16, tag=f"u{i}")
                    tmp = tmp_pool.tile([128, 3 * r * cg], f16, tag=f"t{i}")

                    tr = tc_t[:, 0:12 * cg].rearrange("p (dk c) -> p dk c", c=cg)
                    cr = tc_t[:, 12 * cg:].rearrange("p (j r c) -> p j r c", r=r, c=cg)
                    ur = u[:].rearrange("p (k r c) -> p k r c", r=r, c=cg)
                    tmr = tmp[:].rearrange("p (k r c) -> p k r c", r=r, c=cg)

                    def rows(d):  # trow planes d*3..d*3+3, bcast over r
                        return tr[:, 3 * d:3 * d + 3, :].unsqueeze(2).to_broadcast(
                            [128, 3, r, cg]
                        )

                    def xyz(j):  # cent plane j, bcast over k
                        return cr[:, j:j + 1, :, :].to_broadcast([128, 3, r, cg])

                    nc.vector.tensor_tensor(out=tmr, in0=xyz(0), in1=rows(1), op=mul)
                    nc.vector.tensor_tensor(out=ur, in0=rows(0), in1=tmr, op=add)
                    nc.vector.tensor_tensor(out=tmr, in0=xyz(1), in1=rows(2), op=mul)
                    nc.vector.tensor_tensor(out=ur, in0=ur, in1=tmr, op=add)
                    nc.vector.tensor_tensor(out=tmr, in0=xyz(2), in1=rows(3), op=mul)
                    nc.vector.tensor_tensor(out=ur, in0=ur, in1=tmr, op=add)

                    # stores ride the Activation engine's HWDGE queue so the
                    # next tiles' loads (Sync queue) never wait behind them
                    nc.scalar.dma_start(out=out[t], in_=u[:])
    nc.finalize()
    return nc


_NC_CACHE: dict = {}


def _get_nc():
    if TILES not in _NC_CACHE:
        _NC_CACHE[TILES] = build_nc()
    return _NC_CACHE[TILES]


def _prep_core(el, vox, coeffs16_c, cent16_full):
    """Build one core's device arrays from its (sorted) local element ids."""
    n = el.shape[0]
    L = np.bincount(el, minlength=EPC)
    q, m = L // 8, L % 8
    grp = {"A": q}
    for k in range(1, 8):
        grp[f"M{k}"] = (m == k).astype(np.int64)
    base = {}
    for r in _order:
        b = np.zeros(EPC, dtype=np.int64)
        np.cumsum(grp[r][:-1], out=b[1:])
        base[r] = b
    run_start = np.zeros(EPC, dtype=np.int64)
    np.cumsum(L[:-1], out=run_start[1:])

    rank = np.arange(n, dtype=np.int64) - run_start[el]
    athr = grp["A"][el] * 8              # slots this element owns in A
    in_a = rank < athr
    rem = rank - athr
    m_el = m[el]
    # remainder region index into _order (A, M7..M1): m-k remainder -> 8-k
    ridx = np.where(m_el > 0, 8 - m_el, 0)
    r_arr = np.array([REGION_R[r] for r in _order])
    slot0_arr = np.array([SLOT0[r] for r in _order])
    end_arr = np.array([SLOT0[r] + NS[r] for r in _order])
    base_stack = np.stack([base[r] for r in _order])     # [5, EPC]
    rem_slot = slot0_arr[ridx] + base_stack[ridx, el] * r_arr[ridx] + rem
    slot = np.where(in_a, base["A"][el] * 8 + rank, rem_slot)
    ok = np.where(in_a, slot < NS["A"], slot < end_arr[ridx])

    trow = {}
    for r in _order:
        buf = np.zeros((CAP[r], 12), dtype=np.float16)
        if r == "A":
            rep = np.repeat(coeffs16_c, grp[r], axis=0)
        else:
            rep = coeffs16_c[grp[r].astype(bool)]
        buf[:min(rep.shape[0], CAP[r])] = rep[:CAP[r]]
        trow[r] = buf

    cent_slot = np.zeros((NSLOT, 3), dtype=np.float16)
    cent_slot[slot[ok]] = cent16_full[vox[ok]]

    # slice group-major streams into per-tile-class planar arrays
    gpos = {r: 0 for r in _order}
    in_map = {}
    for i, (rg, nt, cg, _) in enumerate(TILES):
        r = REGION_R[rg]
        ng = nt * 128 * cg
        g0 = gpos[rg]
        rows = trow[rg][g0:g0 + ng]
        s0 = SLOT0[rg] + g0 * r
        cent = cent_slot[s0:s0 + ng * r]
        gpos[rg] = g0 + ng
        trow_p = rows.reshape(nt, 128, cg, 12).transpose(0, 1, 3, 2).reshape(
            nt, 128, 12 * cg
        )
        cent_p = cent.reshape(nt, 128, cg, r, 3).transpose(0, 1, 4, 3, 2).reshape(
            nt, 128, 3 * r * cg
        )
        in_map[f"tc{i}"] = np.ascontiguousarray(
            np.concatenate([trow_p, cent_p], axis=2)
        )

    return in_map, slot, ok


def _reassemble(results_c):
    """Concatenate per-tile outputs back to [NSLOT, 3] in slot order."""
    parts = {r: [] for r in _order}
    for i, (rg, nt, cg, _) in enumerate(TILES):
        r = REGION_R[rg]
        blk = results_c[f"out{i}"].reshape(nt, 128, 3, r, cg)
        parts[rg].append(
            np.ascontiguousarray(blk.transpose(0, 1, 4, 3, 2)).reshape(-1, 3)
        )
    return np.concatenate([p for r in _order for p in parts[r]], axis=0)


def kernel(all_coeffs, all_voxels_centroids, voxels_elements, _trace=False, **run_kwargs):
    nc = _get_nc()
    coeffs12 = np.asarray(all_coeffs, dtype=np.float32).reshape(N_ELEM, 12)
    coeffs16 = coeffs12.astype(np.float16)
    cent_full = np.asarray(all_voxels_centroids, dtype=np.float32)
    cent16 = cent_full.astype(np.float16)
    e_full = np.asarray(voxels_elements).astype(np.int64)

    order = np.argsort(e_full, kind="stable")
    es = e_full[order]
    bounds = np.searchsorted(es, np.arange(N_CORES + 1, dtype=np.int64) * EPC)

    in_maps, metas = [], []
    for c in range(N_CORES):
        lo, hi = int(bounds[c]), int(bounds[c + 1])
        vox = order[lo:hi]
        el = (es[lo:hi] - c * EPC).astype(np.int64)
        m, slot, ok = _prep_core(el, vox, coeffs16[c * EPC:(c + 1) * EPC], cent16)
        in_maps.append(m)
        metas.append((vox, slot, ok))

    res = run_bass_kernel_spmd(
        nc, in_maps, core_ids=list(range(N_CORES)), trace=_trace, **run_kwargs
    )

    full = np.empty((N_VOXELS, 3), dtype=np.float32)
    for c in range(N_CORES):
        vox, slot, ok = metas[c]
        u_slots = _reassemble(res.results[c])
        full[vox[ok]] = u_slots[slot[ok]].astype(np.float32)
        bad = ~ok
        if bad.any():
            vb = vox[bad]
            cf = coeffs12[e_full[vb]].reshape(-1, 4, 3)
            xyz = cent_full[vb]
            full[vb] = cf[:, 0] + np.einsum("nd,ndk->nk", xyz, cf[:, 1:4])
    if _trace:
        return full, res
    return full
